# revision 53
# baseline (speedup 1.0000x reference)
"""Char BiLSTM encoder Trainium2 kernel.

Strategy (data-parallel over 8 cores, 2048 words each):
- Host folds embedding-lookup + input-projection + biases into one fp16 table
  P[d] = char_emb @ w_ih[d].T + b_ih[d] + b_hh[d]  (513 rows; row 512 = poison),
  gathered per (word, step) with dma_gather.  Gate columns are permuted to
  [i, f, o, g] and the g columns pre-doubled so ONE sigmoid covers all gates
  (tanh(g) = 2*sig(2g) - 1, reconstructed with one fused DVE op).
- Masked-scan semantics are implemented with *no mask ops*: the forward
  direction is right-aligned on the host (padded slots first) and the backward
  direction is processed t=15..0, so padded slots always occur while (h,c)=0;
  poison gate values (i=-30, f=+30, g=o=0) keep the state frozen at 0 through
  the padding.
- Per step on-device: PSUM gates = I@xp + hT@whhT (identity-matmul injects the
  gathered projection into the accumulation), one sigmoid over all gates,
  tensor_tensor ops update c and h (fp16, 2x DVE rate), PE-transpose of h
  (fp16 PSUM) + one copy builds the next step's stationary operand.
- Scheduling: two software streams share PSUM — a "pair" stream (both
  directions of a word tile lockstep in a 4-bank slot, activations batched
  across the two chains) and a "duo" stream (two independent single chains
  rotating through a 2-bank slot); blocks of two groups pipeline, gathers
  prefetched ~8 steps ahead.
"""

import os
import numpy as np

os.environ.setdefault("MYCRO_LOCAL_CACHE", "1")

B, W, L = 128, 128, 16
NCHARS, D, H = 512, 100, 200
G = 800          # 4*H gate width
GP = 896         # table row padded to 256B multiple (fp16)
NCORES = 8
NTOT = B * W     # 16384 words
NLOC = NTOT // NCORES  # 2048 words/core
NWT = NLOC // 128      # 16 word tiles/core
TROWS = NCHARS + 1     # 513 rows per direction (512 = poison)
POIS = 30.0            # poison magnitude on gate pre-activations

_GATE_PERM = None


def _gate_perm():
    # original gate order [i, f, g, o] -> new [i, f, o, g]
    global _GATE_PERM
    if _GATE_PERM is None:
        ix = np.arange(G).reshape(4, H)
        _GATE_PERM = np.concatenate([ix[0], ix[1], ix[3], ix[2]])
    return _GATE_PERM


def host_prep(char_ids, char_emb, w_ih_f, w_hh_f, b_ih_f, b_hh_f,
              w_ih_b, w_hh_b, b_ih_b, b_hh_b, n_wt=NWT, n_cores=NCORES):
    """Build device inputs. Returns (in_maps, meta)."""
    ids = np.asarray(char_ids).reshape(NTOT, L).astype(np.int64)
    lens = (ids != 0).sum(-1)
    lens = np.maximum(lens, 1)
    perm = _gate_perm()

    def mk_tab(w_ih, bias):
        P = np.asarray(char_emb, np.float32) @ np.asarray(w_ih, np.float32).T
        P = P + np.asarray(bias, np.float32)[None, :]
        P = P[:, perm]
        P[:, 600:G] *= 2.0
        pois = np.zeros((1, G), np.float32)
        pois[0, 0:H] = -POIS       # i -> sig ~ 0
        pois[0, H:2 * H] = POIS    # f -> sig ~ 1
        P = np.concatenate([P, pois], 0)          # [513, 800]
        out = np.zeros((TROWS, GP), np.float16)
        out[:, :G] = P.astype(np.float16)
        return out

    ptab = np.concatenate(
        [mk_tab(w_ih_f, np.asarray(b_ih_f) + np.asarray(b_hh_f)),
         mk_tab(w_ih_b, np.asarray(b_ih_b) + np.asarray(b_hh_b))], 0)  # [1026, 896]

    def mk_whh(w_hh):
        wt = np.asarray(w_hh, np.float32).T[:, perm].copy()  # [200, 800]
        wt[:, 600:G] *= 2.0
        return wt.astype(np.float16)

    whhT = np.concatenate([mk_whh(w_hh_f), mk_whh(w_hh_b)], 1)  # [200, 1600]

    # index tables ---------------------------------------------------------
    t = np.arange(L)[None, :]
    shift = (L - lens)[:, None]
    src = t - shift
    idx_f = np.where(src >= 0,
                     np.take_along_axis(ids, np.clip(src, 0, L - 1), 1),
                     NCHARS).astype(np.int64)                       # [N, 16]
    tb = (L - 1) - t                                                # orig pos per slot
    idx_b = np.where(tb < lens[:, None], ids[:, ::-1] + TROWS,
                     NCHARS + TROWS).astype(np.int64)               # [N, 16]
    idx_all = np.stack([idx_f, idx_b], 1)                           # [N, 2, 16]

    in_maps = []
    for c in range(n_cores):
        a = idx_all[c * NLOC:c * NLOC + n_wt * 128]                 # [n_wt*128, 2, 16]
        a = a.reshape(n_wt, 128, 2, 2, 8)                           # [wt, w, d, half, sl]
        # flat index i = sl*128 + w  ->  token (w, slot half*8+sl)
        a = a.transpose(0, 2, 3, 4, 1)                              # [wt, d, half, sl, w]
        flat = a.reshape(n_wt, 2, 2, 1024)
        wrapped = flat.reshape(n_wt, 2, 2, 64, 16)                  # [.., c64, p16]
        wrapped = wrapped.transpose(4, 0, 1, 2, 3)                  # [16, wt, d, half, c64]
        g16 = wrapped.reshape(16, n_wt * 2 * 2 * 64).astype(np.int16)
        g128 = np.tile(g16, (8, 1))                                 # [128, cols]
        in_maps.append({"gidx": np.ascontiguousarray(g128),
                        "ptab": ptab, "whhT": whhT})
    return in_maps


# --------------------------------------------------------------------------
# device kernel builder
# --------------------------------------------------------------------------

def build_kernel(tc, outs, ins, n_wt=NWT):
    from concourse import mybir
    from concourse.masks import make_identity
    from contextlib import ExitStack

    F16 = mybir.dt.float16
    F32 = mybir.dt.float32
    I16 = mybir.dt.int16
    TANH = mybir.ActivationFunctionType.Tanh
    SIG = mybir.ActivationFunctionType.Sigmoid
    ADD = mybir.AluOpType.add
    MULT = mybir.AluOpType.mult

    nc = tc.nc
    out_d = outs["out"]
    gidx_d = ins["gidx"]
    ptab_d = ins["ptab"]
    whh_d = ins["whhT"]

    n_sing = 6 if n_wt == 16 else n_wt // 3
    n_pair = n_wt - n_sing

    ctx = ExitStack()
    const = ctx.enter_context(tc.tile_pool(name="const", bufs=1))
    ident = const.tile([128, 128], F16)
    make_identity(nc, ident[:])
    whh_sb = const.tile([100, 3200], F16)  # [100, kc(2) x 1600]
    nc.sync.dma_start(whh_sb[:, 0:1600], whh_d[0:100, :])
    nc.sync.dma_start(whh_sb[:, 1600:3200], whh_d[100:200, :])

    slab_p = ctx.enter_context(tc.tile_pool(name="slabp", bufs=6))
    slab_s = ctx.enter_context(tc.tile_pool(name="slabs", bufs=4))
    gi_pool = ctx.enter_context(tc.tile_pool(name="gip", bufs=3))
    work = ctx.enter_context(tc.tile_pool(name="work", bufs=2))
    state = ctx.enter_context(tc.tile_pool(name="state", bufs=4))
    outp = ctx.enter_context(tc.tile_pool(name="outp", bufs=3))
    pp_pool = ctx.enter_context(tc.tile_pool(name="ppsum", bufs=1, space="PSUM"))
    sp_pool = ctx.enter_context(tc.tile_pool(name="spsum", bufs=1, space="PSUM"))
    tp_pool = ctx.enter_context(tc.tile_pool(name="tpsum", bufs=2, space="PSUM"))

    def whh(kc, d, n0, n1):
        return whh_sb[:, kc * 1600 + d * G + n0: kc * 1600 + d * G + n1]

    class Group:
        """nch lockstep chains: pair = (f,b) of one word tile, single = one."""

        def __init__(self, wt, dirs, kind):
            self.wt, self.dirs, self.kind = wt, dirs, kind
            self.nch = len(dirs)
            self.out_stage = None
            self.hT = None
            self.c2 = None
            self.alloc_stage = lambda: outp.tile([128, 2 * H], F32, tag="outs", name="outs")

        def start_gi(self):
            gbase = self.wt * 2 * 2 * 64
            gi = gi_pool.tile([128, self.nch * 2 * 64], I16,
                              tag="gi" + self.kind, name="gi")
            if self.nch == 2:
                nc.sync.dma_start(gi[:], gidx_d[:, gbase:gbase + 256])
            else:
                d = self.dirs[0]
                nc.sync.dma_start(
                    gi[:], gidx_d[:, gbase + d * 128: gbase + d * 128 + 128])
            self.gi = gi
            self.slabs = {}

        def gather(self, half):
            pool = slab_p if self.kind == "p" else slab_s
            for ci, d in enumerate(self.dirs):
                tl = pool.tile([128, 8 * GP], F16, tag="slab" + self.kind,
                               name="slab")
                t3 = tl[:].rearrange("p (h x) -> p h x", h=8)
                for q in range(2):
                    nc.gpsimd.dma_gather(
                        t3[:, q * 4:(q + 1) * 4, :],
                        ptab_d[:, :],
                        self.gi[:, (ci * 2 + half) * 64 + q * 32:
                                (ci * 2 + half) * 64 + (q + 1) * 32],
                        512, 512, GP,
                        queue_num=0,
                    )
                self.slabs[(d, half)] = tl

        def step(self, s):
            nch = self.nch
            half, sl = divmod(s, 8)
            if s == 0:
                tb = work.tile([128, nch * G], F16, tag="T" + self.kind)
                for ci, d in enumerate(self.dirs):
                    sb = self.slabs[(d, 0)]
                    nc.scalar.activation(tb[:, ci * G:(ci + 1) * G],
                                         sb[:, 0:G], SIG)
            else:
                pp = (pp_pool if nch == 2 else sp_pool).tile(
                    [128, nch * 1024], F32, tag="pp" + self.kind)
                for ci, d in enumerate(self.dirs):
                    sb = self.slabs[(d, half)]
                    o = ci * 1024
                    for (n0, n1) in ((0, 512), (512, G)):
                        nc.tensor.matmul(pp[:, o + n0:o + n1], ident[:],
                                         sb[:, sl * GP + n0: sl * GP + n1],
                                         start=True, stop=False)
                        nc.tensor.matmul(pp[:, o + n0:o + n1],
                                         self.hT[:, ci * 256: ci * 256 + 128],
                                         whh(0, d, n0, n1),
                                         start=False, stop=False)
                        nc.tensor.matmul(pp[:, o + n0:o + n1],
                                         self.hT[:, ci * 256 + 128: ci * 256 + 256],
                                         whh(1, d, n0, n1),
                                         start=False, stop=True)
                tb = work.tile([128, nch * G], F16, tag="T" + self.kind)
                nc.scalar.activation(
                    tb[:].rearrange("p (c x) -> p c x", c=nch),
                    pp[:].rearrange("p (c x) -> p c x", c=nch)[:, :, 0:G],
                    SIG)

            t3 = tb[:].rearrange("p (c x) -> p c x", c=nch)
            if s == 0:
                self.c2 = state.tile([128, nch * H], F16,
                                     tag="c2" + self.kind, name="c2")
            c2v = self.c2[:].rearrange("p (c x) -> p c x", c=nch)
            SUB = mybir.AluOpType.subtract
            m = work.tile([128, nch * H], F16, tag="m" + self.kind)
            mv = m[:].rearrange("p (c x) -> p c x", c=nch)
            nc.vector.tensor_tensor(
                mv, t3[:, :, 0:H], t3[:, :, 600:G], op=MULT)
            if s == 0:
                # c = si * tanh(g) = 2*si*sg2 - si
                nc.vector.scalar_tensor_tensor(
                    c2v, mv, 2.0, t3[:, :, 0:H], MULT, SUB)
            else:
                u = work.tile([128, nch * H], F16, tag="u" + self.kind)
                uv = u[:].rearrange("p (c x) -> p c x", c=nch)
                nc.vector.scalar_tensor_tensor(
                    uv, mv, 2.0, t3[:, :, 0:H], MULT, SUB)
                fc = work.tile([128, nch * H], F16, tag="fc" + self.kind)
                fcv = fc[:].rearrange("p (c x) -> p c x", c=nch)
                nc.vector.tensor_tensor(
                    fcv, t3[:, :, H:2 * H], c2v, op=MULT)
                nc.vector.tensor_tensor(c2v, fcv, uv, op=ADD)
            th = work.tile([128, nch * H], F16, tag="th" + self.kind)
            nc.scalar.activation(th[:], self.c2[:], TANH)
            thv = th[:].rearrange("p (c x) -> p c x", c=nch)

            if s < L - 1:
                h2 = work.tile([128, nch * H], F16, tag="h2" + self.kind)
                h2v = h2[:].rearrange("p (c x) -> p c x", c=nch)
                nc.vector.tensor_tensor(
                    h2v, t3[:, :, 2 * H:600], thv, op=MULT)
                tp = tp_pool.tile([100, nch * 256], F16, tag="tp")
                for ci in range(nch):
                    for kc in range(2):
                        nc.tensor.transpose(
                            tp[:, ci * 256 + kc * 128: ci * 256 + (kc + 1) * 128],
                            h2[:, ci * H + kc * 100: ci * H + (kc + 1) * 100],
                            ident[:])
                hT = state.tile([100, nch * 256], F16, tag="hT" + self.kind)
                nc.vector.tensor_copy(hT[:], tp[:])
                self.hT = hT
            else:
                if self.out_stage is None:
                    self.out_stage = self.alloc_stage()
                ov = self.out_stage[:].rearrange("p (c x) -> p c x", c=2)
                if nch == 2:
                    tgt = ov
                else:
                    d = self.dirs[0]
                    tgt = ov[:, d:d + 1, :]
                nc.vector.tensor_tensor(
                    tgt, t3[:, :, 2 * H:600], thv, op=MULT)

        def finish(self):
            nc.sync.dma_start(
                out_d[self.wt * 128:(self.wt + 1) * 128, :], self.out_stage[:])

    def pair_stream():
        wts = list(range(n_pair))
        groups = [[Group(wt, (0, 1), "p") for wt in wts[i:i + 2]]
                  for i in range(0, len(wts), 2)]
        for g in groups[0]:
            g.start_gi()
            g.gather(0)
            g.gather(1)
        for bi, blk in enumerate(groups):
            nxt = groups[bi + 1] if bi + 1 < len(groups) else None
            for s in range(L):
                for g in blk:
                    g.step(s)
                    yield
                if nxt and s == 5:
                    for g in nxt:
                        g.start_gi()
                        g.gather(0)
                elif nxt and s == 11:
                    for g in nxt:
                        g.gather(1)
            for g in blk:
                g.finish()

    def single_stream():
        stage = {}
        chains = []
        for wt in range(n_pair, n_wt):
            for d in (0, 1):
                g = Group(wt, (d,), "s")
                if d == 0:
                    g.alloc_stage = lambda wt=wt: stage.setdefault(
                        wt, outp.tile([128, 2 * H], F32, tag="outs", name="outs"))
                else:
                    g.alloc_stage = lambda wt=wt: stage[wt]
                chains.append(g)
        duos = [chains[i:i + 2] for i in range(0, len(chains), 2)]
        for g in duos[0]:
            g.start_gi()
            g.gather(0)
        for di, duo in enumerate(duos):
            nxt = duos[di + 1] if di + 1 < len(duos) else None
            for s in range(L):
                for g in duo:
                    g.step(s)
                    yield
                if s == 4:
                    for g in duo:
                        g.gather(1)
                elif nxt and s == 11:
                    for g in nxt:
                        g.start_gi()
                        g.gather(0)
            for g in duo:
                if g.dirs[0] == 1:
                    g.finish()

    streams = [pair_stream()]
    if n_sing:
        streams.append(single_stream())
    while streams:
        for st in list(streams):
            try:
                next(st)
            except StopIteration:
                streams.remove(st)
    ctx.close()


# --------------------------------------------------------------------------
# host entry
# --------------------------------------------------------------------------

_CACHE = {}


def _get_program(n_wt=NWT):
    key = n_wt
    if key not in _CACHE:
        import concourse.bacc as bacc
        import concourse.tile as tile
        from concourse import mybir

        nc = bacc.Bacc("TRN2", target_bir_lowering=False, debug=False,
                       enable_asserts=False, num_swdge_queues=1,
                       dynamic_dma_scratch_size=32768)
        gidx = nc.dram_tensor("gidx", [128, n_wt * 256], mybir.dt.int16,
                              kind="ExternalInput").ap()
        ptab = nc.dram_tensor("ptab", [2 * TROWS, GP], mybir.dt.float16,
                              kind="ExternalInput").ap()
        whhT = nc.dram_tensor("whhT", [H, 1600], mybir.dt.float16,
                              kind="ExternalInput").ap()
        out = nc.dram_tensor("out", [n_wt * 128, 2 * H], mybir.dt.float32,
                             kind="ExternalOutput").ap()
        with tile.TileContext(nc) as tc:
            build_kernel(tc, {"out": out}, {"gidx": gidx, "ptab": ptab,
                                            "whhT": whhT}, n_wt=n_wt)
        nc.compile()
        _CACHE[key] = nc
    return _CACHE[key]


def run(inputs, trace=False, n_wt=NWT, n_cores=NCORES):
    from concourse.bass_utils import run_bass_kernel_spmd

    in_maps = host_prep(**inputs, n_wt=n_wt, n_cores=n_cores)
    nc = _get_program(n_wt)
    res = run_bass_kernel_spmd(nc, in_maps, core_ids=list(range(n_cores)),
                               trace=trace)
    outs = np.concatenate([r["out"] for r in res.results], 0)
    return outs, res


def kernel(**inputs):
    outs, _ = run(inputs)
    return outs.reshape(B, W, 2 * H).astype(np.float32)


# revision 58
# speedup vs baseline: 1.0482x; 1.0482x over previous
"""Char BiLSTM encoder Trainium2 kernel.

Strategy (data-parallel over 8 cores, 2048 words each):
- Host folds embedding-lookup + input-projection + biases into one fp16 table
  P[d] = char_emb @ w_ih[d].T + b_ih[d] + b_hh[d]  (513 rows; row 512 = poison),
  gathered per (word, step) with dma_gather.  Gate columns are permuted to
  [i, f, o, g] and the g columns pre-doubled so ONE sigmoid covers all gates
  (tanh(g) = 2*sig(2g) - 1, reconstructed with one fused DVE op).
- Masked-scan semantics are implemented with *no mask ops*: the forward
  direction is right-aligned on the host (padded slots first) and the backward
  direction is processed t=15..0, so padded slots always occur while (h,c)=0;
  poison gate values (i=-30, f=+30, g=o=0) keep the state frozen at 0 through
  the padding.
- Per step on-device: PSUM gates = I@xp + hT@whhT (identity-matmul injects the
  gathered projection into the accumulation), one sigmoid over all gates,
  tensor_tensor ops update c and h (fp16, 2x DVE rate), PE-transpose of h
  (fp16 PSUM) + one copy builds the next step's stationary operand.
- Scheduling: two software streams share PSUM — a "pair" stream (both
  directions of a word tile lockstep in a 4-bank slot, activations batched
  across the two chains) and a "duo" stream (two independent single chains
  rotating through a 2-bank slot); blocks of two groups pipeline, gathers
  prefetched ~8 steps ahead.
"""

import os
import numpy as np

os.environ.setdefault("MYCRO_LOCAL_CACHE", "1")

B, W, L = 128, 128, 16
NCHARS, D, H = 512, 100, 200
G = 800          # 4*H gate width
GP = 896         # table row padded to 256B multiple (fp16)
NCORES = 8
NTOT = B * W     # 16384 words
NLOC = NTOT // NCORES  # 2048 words/core
NWT = NLOC // 128      # 16 word tiles/core
TROWS = NCHARS + 1     # 513 rows per direction (512 = poison)
POIS = 30.0            # poison magnitude on gate pre-activations

_GATE_PERM = None


def _gate_perm():
    # original gate order [i, f, g, o] -> new [i, f, o, g]
    global _GATE_PERM
    if _GATE_PERM is None:
        ix = np.arange(G).reshape(4, H)
        _GATE_PERM = np.concatenate([ix[0], ix[1], ix[3], ix[2]])
    return _GATE_PERM


def host_prep(char_ids, char_emb, w_ih_f, w_hh_f, b_ih_f, b_hh_f,
              w_ih_b, w_hh_b, b_ih_b, b_hh_b, n_wt=NWT, n_cores=NCORES):
    """Build device inputs. Returns (in_maps, meta)."""
    ids = np.asarray(char_ids).reshape(NTOT, L).astype(np.int64)
    lens = (ids != 0).sum(-1)
    lens = np.maximum(lens, 1)
    perm = _gate_perm()

    def mk_tab(w_ih, bias):
        P = np.asarray(char_emb, np.float32) @ np.asarray(w_ih, np.float32).T
        P = P + np.asarray(bias, np.float32)[None, :]
        P = P[:, perm]
        P[:, 600:G] *= 2.0
        pois = np.zeros((1, G), np.float32)
        pois[0, 0:H] = -POIS       # i -> sig ~ 0
        pois[0, H:2 * H] = POIS    # f -> sig ~ 1
        P = np.concatenate([P, pois], 0)          # [513, 800]
        out = np.zeros((TROWS, GP), np.float16)
        out[:, :G] = P.astype(np.float16)
        return out

    ptab = np.concatenate(
        [mk_tab(w_ih_f, np.asarray(b_ih_f) + np.asarray(b_hh_f)),
         mk_tab(w_ih_b, np.asarray(b_ih_b) + np.asarray(b_hh_b))], 0)  # [1026, 896]

    def mk_whh(w_hh):
        wt = np.asarray(w_hh, np.float32).T[:, perm].copy()  # [200, 800]
        wt[:, 600:G] *= 2.0
        return wt.astype(np.float16)

    whhT = np.concatenate([mk_whh(w_hh_f), mk_whh(w_hh_b)], 1)  # [200, 1600]

    # index tables ---------------------------------------------------------
    t = np.arange(L)[None, :]
    shift = (L - lens)[:, None]
    src = t - shift
    idx_f = np.where(src >= 0,
                     np.take_along_axis(ids, np.clip(src, 0, L - 1), 1),
                     NCHARS).astype(np.int64)                       # [N, 16]
    tb = (L - 1) - t                                                # orig pos per slot
    idx_b = np.where(tb < lens[:, None], ids[:, ::-1] + TROWS,
                     NCHARS + TROWS).astype(np.int64)               # [N, 16]
    idx_all = np.stack([idx_f, idx_b], 1)                           # [N, 2, 16]

    in_maps = []
    for c in range(n_cores):
        a = idx_all[c * NLOC:c * NLOC + n_wt * 128]                 # [n_wt*128, 2, 16]
        a = a.reshape(n_wt, 128, 2, 2, 8)                           # [wt, w, d, half, sl]
        # flat index i = sl*128 + w  ->  token (w, slot half*8+sl)
        a = a.transpose(0, 2, 3, 4, 1)                              # [wt, d, half, sl, w]
        flat = a.reshape(n_wt, 2, 2, 1024)
        wrapped = flat.reshape(n_wt, 2, 2, 64, 16)                  # [.., c64, p16]
        wrapped = wrapped.transpose(4, 0, 1, 2, 3)                  # [16, wt, d, half, c64]
        g16 = wrapped.reshape(16, n_wt * 2 * 2 * 64).astype(np.int16)
        g128 = np.tile(g16, (8, 1))                                 # [128, cols]
        in_maps.append({"gidx": np.ascontiguousarray(g128),
                        "ptab": ptab, "whhT": whhT})
    return in_maps


# --------------------------------------------------------------------------
# device kernel builder
# --------------------------------------------------------------------------

def build_kernel(tc, outs, ins, n_wt=NWT):
    from concourse import mybir
    from concourse.masks import make_identity
    from contextlib import ExitStack

    F16 = mybir.dt.float16
    F32 = mybir.dt.float32
    I16 = mybir.dt.int16
    TANH = mybir.ActivationFunctionType.Tanh
    SIG = mybir.ActivationFunctionType.Sigmoid
    ADD = mybir.AluOpType.add
    MULT = mybir.AluOpType.mult

    nc = tc.nc
    out_d = outs["out"]
    gidx_d = ins["gidx"]
    ptab_d = ins["ptab"]
    whh_d = ins["whhT"]

    n_sing = 6 if n_wt == 16 else n_wt // 3
    n_pair = n_wt - n_sing

    ctx = ExitStack()
    const = ctx.enter_context(tc.tile_pool(name="const", bufs=1))
    ident = const.tile([128, 128], F16)
    make_identity(nc, ident[:])
    whh_sb = const.tile([100, 3200], F16)  # [100, kc(2) x 1600]
    nc.sync.dma_start(whh_sb[:, 0:1600], whh_d[0:100, :])
    nc.sync.dma_start(whh_sb[:, 1600:3200], whh_d[100:200, :])

    slab_p = ctx.enter_context(tc.tile_pool(name="slabp", bufs=6))
    slab_s = ctx.enter_context(tc.tile_pool(name="slabs", bufs=4))
    gi_pool = ctx.enter_context(tc.tile_pool(name="gip", bufs=3))
    work = ctx.enter_context(tc.tile_pool(name="work", bufs=2))
    state = ctx.enter_context(tc.tile_pool(name="state", bufs=4))
    outp = ctx.enter_context(tc.tile_pool(name="outp", bufs=3))
    pp_pool = ctx.enter_context(tc.tile_pool(name="ppsum", bufs=1, space="PSUM"))
    sp_pool = ctx.enter_context(tc.tile_pool(name="spsum", bufs=1, space="PSUM"))
    tp_pool = ctx.enter_context(tc.tile_pool(name="tpsum", bufs=2, space="PSUM"))

    def whh(kc, d, n0, n1):
        return whh_sb[:, kc * 1600 + d * G + n0: kc * 1600 + d * G + n1]

    class Group:
        """nch lockstep chains: pair = (f,b) of one word tile, single = one."""

        def __init__(self, wt, dirs, kind):
            self.wt, self.dirs, self.kind = wt, dirs, kind
            self.nch = len(dirs)
            self.out_stage = None
            self.hT = None
            self.c2 = None
            self.alloc_stage = lambda: outp.tile([128, 2 * H], F32, tag="outs", name="outs")

        def start_gi(self):
            gbase = self.wt * 2 * 2 * 64
            gi = gi_pool.tile([128, self.nch * 2 * 64], I16,
                              tag="gi" + self.kind, name="gi")
            if self.nch == 2:
                nc.sync.dma_start(gi[:], gidx_d[:, gbase:gbase + 256])
            else:
                d = self.dirs[0]
                nc.sync.dma_start(
                    gi[:], gidx_d[:, gbase + d * 128: gbase + d * 128 + 128])
            self.gi = gi
            self.slabs = {}

        def gather(self, half):
            pool = slab_p if self.kind == "p" else slab_s
            for ci, d in enumerate(self.dirs):
                tl = pool.tile([128, 8 * GP], F16, tag="slab" + self.kind,
                               name="slab")
                t3 = tl[:].rearrange("p (h x) -> p h x", h=8)
                for q in range(2):
                    nc.gpsimd.dma_gather(
                        t3[:, q * 4:(q + 1) * 4, :],
                        ptab_d[:, :],
                        self.gi[:, (ci * 2 + half) * 64 + q * 32:
                                (ci * 2 + half) * 64 + (q + 1) * 32],
                        512, 512, GP,
                        queue_num=0,
                    )
                self.slabs[(d, half)] = tl

        def step(self, s):
            nch = self.nch
            half, sl = divmod(s, 8)
            if s == 0:
                tb = work.tile([128, nch * G], F16, tag="T" + self.kind)
                for ci, d in enumerate(self.dirs):
                    sb = self.slabs[(d, 0)]
                    nc.scalar.activation(tb[:, ci * G:(ci + 1) * G],
                                         sb[:, 0:G], SIG)
            else:
                pp = (pp_pool if nch == 2 else sp_pool).tile(
                    [128, nch * 1024], F32, tag="pp" + self.kind)
                for ci, d in enumerate(self.dirs):
                    sb = self.slabs[(d, half)]
                    o = ci * 1024
                    for (n0, n1) in ((0, 512), (512, G)):
                        nc.tensor.matmul(pp[:, o + n0:o + n1], ident[:],
                                         sb[:, sl * GP + n0: sl * GP + n1],
                                         start=True, stop=False)
                        nc.tensor.matmul(pp[:, o + n0:o + n1],
                                         self.hT[:, ci * 256: ci * 256 + 128],
                                         whh(0, d, n0, n1),
                                         start=False, stop=False)
                        nc.tensor.matmul(pp[:, o + n0:o + n1],
                                         self.hT[:, ci * 256 + 128: ci * 256 + 256],
                                         whh(1, d, n0, n1),
                                         start=False, stop=True)
                tb = work.tile([128, nch * G], F16, tag="T" + self.kind)
                nc.scalar.activation(
                    tb[:].rearrange("p (c x) -> p c x", c=nch),
                    pp[:].rearrange("p (c x) -> p c x", c=nch)[:, :, 0:G],
                    SIG)

            t3 = tb[:].rearrange("p (c x) -> p c x", c=nch)
            if s == 0:
                self.c2 = state.tile([128, nch * H], F16,
                                     tag="c2" + self.kind, name="c2")
            c2v = self.c2[:].rearrange("p (c x) -> p c x", c=nch)
            SUB = mybir.AluOpType.subtract
            m = work.tile([128, nch * H], F16, tag="m" + self.kind)
            mv = m[:].rearrange("p (c x) -> p c x", c=nch)
            nc.vector.tensor_tensor(
                mv, t3[:, :, 0:H], t3[:, :, 600:G], op=MULT)
            if s == 0:
                # c = si * tanh(g) = 2*si*sg2 - si
                nc.vector.scalar_tensor_tensor(
                    c2v, mv, 2.0, t3[:, :, 0:H], MULT, SUB)
            else:
                u = work.tile([128, nch * H], F16, tag="u" + self.kind)
                uv = u[:].rearrange("p (c x) -> p c x", c=nch)
                nc.vector.scalar_tensor_tensor(
                    uv, mv, 2.0, t3[:, :, 0:H], MULT, SUB)
                fc = work.tile([128, nch * H], F16, tag="fc" + self.kind)
                fcv = fc[:].rearrange("p (c x) -> p c x", c=nch)
                nc.vector.tensor_tensor(
                    fcv, t3[:, :, H:2 * H], c2v, op=MULT)
                nc.vector.tensor_tensor(c2v, fcv, uv, op=ADD)
            th = work.tile([128, nch * H], F16, tag="th" + self.kind)
            nc.scalar.activation(th[:], self.c2[:], TANH)
            thv = th[:].rearrange("p (c x) -> p c x", c=nch)

            if s < L - 1:
                h2 = work.tile([128, nch * H], F16, tag="h2" + self.kind)
                h2v = h2[:].rearrange("p (c x) -> p c x", c=nch)
                nc.vector.tensor_tensor(
                    h2v, t3[:, :, 2 * H:600], thv, op=MULT)
                tp = tp_pool.tile([100, nch * 256], F16, tag="tp")
                for ci in range(nch):
                    for kc in range(2):
                        nc.tensor.transpose(
                            tp[:, ci * 256 + kc * 128: ci * 256 + (kc + 1) * 128],
                            h2[:, ci * H + kc * 100: ci * H + (kc + 1) * 100],
                            ident[:])
                hT = state.tile([100, nch * 256], F16, tag="hT" + self.kind)
                nc.vector.tensor_copy(hT[:], tp[:])
                self.hT = hT
            else:
                if self.out_stage is None:
                    self.out_stage = self.alloc_stage()
                ov = self.out_stage[:].rearrange("p (c x) -> p c x", c=2)
                if nch == 2:
                    tgt = ov
                else:
                    d = self.dirs[0]
                    tgt = ov[:, d:d + 1, :]
                nc.vector.tensor_tensor(
                    tgt, t3[:, :, 2 * H:600], thv, op=MULT)

        def finish(self):
            nc.sync.dma_start(
                out_d[self.wt * 128:(self.wt + 1) * 128, :], self.out_stage[:])

    def pair_stream():
        wts = list(range(n_pair))
        groups = [[Group(wt, (0, 1), "p") for wt in wts[i:i + 2]]
                  for i in range(0, len(wts), 2)]
        for g in groups[0]:
            g.start_gi()
            g.gather(0)
            g.gather(1)
        for bi, blk in enumerate(groups):
            nxt = groups[bi + 1] if bi + 1 < len(groups) else None
            for s in range(L):
                for g in blk:
                    g.step(s)
                    yield
                if nxt and s == 5:
                    for g in nxt:
                        g.start_gi()
                        g.gather(0)
                elif nxt and s == 11:
                    for g in nxt:
                        g.gather(1)
            for g in blk:
                g.finish()

    def single_stream():
        stage = {}
        chains = []
        for wt in range(n_pair, n_wt):
            for d in (0, 1):
                g = Group(wt, (d,), "s")
                if d == 0:
                    g.alloc_stage = lambda wt=wt: stage.setdefault(
                        wt, outp.tile([128, 2 * H], F32, tag="outs", name="outs"))
                else:
                    g.alloc_stage = lambda wt=wt: stage[wt]
                chains.append(g)
        duos = [chains[i:i + 2] for i in range(0, len(chains), 2)]
        for g in duos[0]:
            g.start_gi()
            g.gather(0)
        for di, duo in enumerate(duos):
            nxt = duos[di + 1] if di + 1 < len(duos) else None
            for s in range(L):
                for g in duo:
                    g.step(s)
                    yield
                if s == 4:
                    for g in duo:
                        g.gather(1)
                elif nxt and s == 11:
                    for g in nxt:
                        g.start_gi()
                        g.gather(0)
            for g in duo:
                if g.dirs[0] == 1:
                    g.finish()

    ps = pair_stream()
    ss = single_stream() if n_sing else None
    if ss is not None:
        # head-start the duo stream so both streams end together and the
        # tail runs on the wider pair stream
        for _ in range(28):
            try:
                next(ss)
            except StopIteration:
                ss = None
                break
    streams = [st for st in (ps, ss) if st is not None]
    while streams:
        for st in list(streams):
            try:
                next(st)
            except StopIteration:
                streams.remove(st)
    ctx.close()


# --------------------------------------------------------------------------
# host entry
# --------------------------------------------------------------------------

_CACHE = {}


def _get_program(n_wt=NWT):
    key = n_wt
    if key not in _CACHE:
        import concourse.bacc as bacc
        import concourse.tile as tile
        from concourse import mybir

        nc = bacc.Bacc("TRN2", target_bir_lowering=False, debug=False,
                       enable_asserts=False, num_swdge_queues=1,
                       dynamic_dma_scratch_size=32768)
        gidx = nc.dram_tensor("gidx", [128, n_wt * 256], mybir.dt.int16,
                              kind="ExternalInput").ap()
        ptab = nc.dram_tensor("ptab", [2 * TROWS, GP], mybir.dt.float16,
                              kind="ExternalInput").ap()
        whhT = nc.dram_tensor("whhT", [H, 1600], mybir.dt.float16,
                              kind="ExternalInput").ap()
        out = nc.dram_tensor("out", [n_wt * 128, 2 * H], mybir.dt.float32,
                             kind="ExternalOutput").ap()
        with tile.TileContext(nc) as tc:
            build_kernel(tc, {"out": out}, {"gidx": gidx, "ptab": ptab,
                                            "whhT": whhT}, n_wt=n_wt)
        nc.compile()
        _CACHE[key] = nc
    return _CACHE[key]


def run(inputs, trace=False, n_wt=NWT, n_cores=NCORES):
    from concourse.bass_utils import run_bass_kernel_spmd

    in_maps = host_prep(**inputs, n_wt=n_wt, n_cores=n_cores)
    nc = _get_program(n_wt)
    res = run_bass_kernel_spmd(nc, in_maps, core_ids=list(range(n_cores)),
                               trace=trace)
    outs = np.concatenate([r["out"] for r in res.results], 0)
    return outs, res


def kernel(**inputs):
    outs, _ = run(inputs)
    return outs.reshape(B, W, 2 * H).astype(np.float32)
